# revision 34
# baseline (speedup 1.0000x reference)
"""Trainium2 Bass kernel: cache-distance -> exp kernel -> vocab histogram -> log_softmax.

Math (per cache row i): kern_i = exp(||cache_h[i] - h_t|| / 0.2)
                        cache_p[v] = sum_{i: word_ids[i]==v} kern_i
                        out = log_softmax(cache_p)[None, :]

Device strategy (8 cores, cache sharded along N, DMA-roofline driven):
  - host sorts cache rows by word_id, so the vocab histogram becomes a
    segment-sum over consecutive elements; uploads the cache slice
    pre-transposed [D=512, 32768] in f16 (halves HBM traffic; validated
    numerically: f16-input rel err 2.8e-4 vs the 2e-2 gate)
  - squared distance via elementwise squares, split between two engines
    to stay under the DMA shadow (ACT 1x and DVE 4x/2x two-pass rates are
    ~balanced at a 13/19 quarter split):
      ACT quarters: (x + (-h))^2 via Square activation w/ per-partition bias
      DVE quarters: y = x + (-h) (tensor_scalar, 4x mode) then z = y*y
        (tensor_tensor, 2x mode) -- scalar_tensor_tensor would be one pass
        but only has a 1x uop (4480ns measured vs 3321ns for the pair)
  - PE reduces over D with one-hot-column [128,16] lhsT matmuls into a
    [16, 512] PSUM dist tile per CHUNK PAIR (2 x 4096 rows; pairing halves
    the ACT kern-chain instruction count and gives the software pipeline a
    full pair of PE work between cross-engine dependency edges)
  - ACT: kern = exp(exp(0.5*ln(25*d2)))  (Ln+Exp+Square share one table
    set; avoids Sqrt)
  - PE transposes kern into [128, 64] batch-major layout, then ONE
    lower-triangular [128,128] matmul per pair produces within-batch
    (128-element) inclusive prefix sums -- the entire scatter reduced to
    4 tiny matmuls per core
Host: combine per-core prefix tiles (f64 batch-offset cumsum), segment
  diffs at sorted-vocab boundaries, log_softmax (tiny, O(V)).
"""

import os
import sys

for _p in ("/root/.axon_site", "/root/.axon_site/_ro/trn_rl_repo",
           "/root/.axon_site/_ro/pypackages"):
    if os.path.isdir(_p) and _p not in sys.path:
        sys.path.append(_p)

import numpy as np

VOCAB = 50257
N_CACHE = 262144
D = 512
SMOOTH = 0.2
NCORES = 8
RPC = N_CACHE // NCORES        # 32768 rows per core
NCHUNK = 8
CHUNK = RPC // NCHUNK          # 4096 rows per chunk
BPC = RPC // 128               # 256 batches of 128 elements per core

_CACHE = {}


def _act_owns(ch, c):
    """13 of 32 (ch, c) quarter-tiles squared on ACT, the rest on DVE --
    balances ACT (1x + kern chain) against DVE (4x+2x two-pass). The last
    chunk gives ACT c=2 so the final DVE tile (c=3) squares in parallel
    with ACT instead of queueing behind a serial DVE backlog."""
    return c == 0 or (c == 2 and ch % 2 == 0) or (ch == 7 and c == 2)


def _patch_act_tables():
    """Restrict the activation table-set chooser to
    natural_log_exp_and_others (covers square/ln/exp/copy) so the whole
    kernel needs exactly one ACT_TABLE_LOAD instead of alternating between
    sets (~2.7us per reload). Set names/order are preserved so
    act_func_set_id indices stay valid."""
    import concourse.hw_specs as hw_specs
    import concourse.bacc as bacc

    if getattr(hw_specs.get_activation_tables, "_histkernel_patched", False):
        return
    orig = hw_specs.get_activation_tables

    def patched(module_arch):
        tabs = orig(module_arch)
        return {
            name: (fns if name == "natural_log_exp_and_others" else set())
            for name, fns in tabs.items()
        }

    patched._histkernel_patched = True
    hw_specs.get_activation_tables = patched
    bacc.get_activation_tables = patched


def _build_program():
    import concourse.bacc as bacc
    import concourse.tile as tile
    import concourse.mybir as mybir

    _patch_act_tables()

    f32, f16 = mybir.dt.float32, mybir.dt.float16
    AF = mybir.ActivationFunctionType
    ALU = mybir.AluOpType

    nc = bacc.Bacc("TRN2", target_bir_lowering=False, debug=False,
                   num_devices=NCORES)

    # tile-major layout: rows [t*128, (t+1)*128) hold quarter-tile
    # t = (ch, c)'s [128, 4096] block, so every DMA reads 1 MiB of
    # fully CONTIGUOUS DRAM (128 lines x 8KB, line stride == line size)
    xt_d = nc.dram_tensor("xt", [32 * 128, CHUNK], f16,
                          kind="ExternalInput")
    nh_d = nc.dram_tensor("nh", [128, 4], f32, kind="ExternalInput")
    oh_d = nc.dram_tensor("oh", [128, 256], f16, kind="ExternalInput")
    id_d = nc.dram_tensor("idm", [16, 16], f32, kind="ExternalInput")
    ltr_d = nc.dram_tensor("ltr", [128, 128], f32, kind="ExternalInput")
    pfx_d = nc.dram_tensor("pfx", [128, BPC], f32, kind="ExternalOutput")

    with tile.TileContext(nc) as tc:
        with (
            tc.tile_pool(name="const", bufs=1) as cpool,
            tc.tile_pool(name="x", bufs=14) as xpool,
            tc.tile_pool(name="sq", bufs=5) as sqpool,
            tc.tile_pool(name="y", bufs=3) as ypool,
            tc.tile_pool(name="s", bufs=4) as spool,
            tc.tile_pool(name="kt", bufs=2) as ktpool,
            tc.tile_pool(name="out", bufs=1) as opool,
            tc.tile_pool(name="psdist", bufs=4, space="PSUM") as psdist,
            tc.tile_pool(name="pskt", bufs=2, space="PSUM") as pskt,
            tc.tile_pool(name="pspfx", bufs=2, space="PSUM") as pspfx,
        ):
            # constants ride the scalar-engine HWDGE queue so the x-tile
            # stream on the sync queue starts issuing at t=0
            nh = cpool.tile([128, 4], f32)
            nc.scalar.dma_start(nh[:], nh_d.ap())
            oh = cpool.tile([128, 256], f16)
            nc.scalar.dma_start(oh[:], oh_d.ap())
            idm = cpool.tile([16, 16], f32)
            nc.scalar.dma_start(idm[:], id_d.ap())
            ltr = cpool.tile([128, 128], f32)
            nc.scalar.dma_start(ltr[:], ltr_d.ap())

            out_sb = opool.tile([128, BPC], f32)
            xt_ap = xt_d.ap()
            pfx_ap = pfx_d.ap()

            def emit_dist_pair(p):
                dist = psdist.tile([16, 512], f32)
                mm = 0
                for sub in range(2):
                    ch = 2 * p + sub
                    for c in range(4):
                        t = ch * 4 + c
                        x = xpool.tile([128, CHUNK], f16)
                        z = sqpool.tile([128, CHUNK], f16)
                        # the very last tile gates the kernel tail: split
                        # it in half so squaring overlaps its own DMA
                        H = CHUNK // 2
                        halves = ((0, H), (H, CHUNK)) if t == 31 \
                            else ((0, CHUNK),)
                        for lo, hi in halves:
                            nc.sync.dma_start(
                                x[:, lo:hi],
                                xt_ap[t * 128:(t + 1) * 128, lo:hi])
                        for hidx, (lo, hi) in enumerate(halves):
                            # split tile: ACT takes half 0, DVE half 1,
                            # so both halves square concurrently
                            on_act = (_act_owns(ch, c) if len(halves) == 1
                                      else hidx == 0)
                            if on_act:
                                nc.scalar.activation(
                                    z[:, lo:hi], x[:, lo:hi], AF.Square,
                                    bias=nh[:, c:c + 1])
                            else:
                                y = ypool.tile([128, CHUNK], f16)
                                nc.vector.tensor_scalar(
                                    y[:, lo:hi], x[:, lo:hi],
                                    nh[:, c:c + 1], None, ALU.add)
                                nc.vector.tensor_tensor(
                                    z[:, lo:hi], y[:, lo:hi], y[:, lo:hi],
                                    ALU.mult)
                        for g in range(8):
                            u = sub * 8 + g
                            nc.tensor.matmul(
                                dist[:],
                                oh[:, u * 16:(u + 1) * 16],
                                z[:, g * 512:(g + 1) * 512],
                                start=(mm == 0),
                                stop=(mm == 63),
                            )
                            mm += 1
                return dist

            def emit_post(p, dist):
                lg = spool.tile([16, 512], f32)
                nc.scalar.activation(lg[:], dist[:], AF.Ln, scale=25.0)
                d5 = spool.tile([16, 512], f32)
                nc.scalar.activation(d5[:], lg[:], AF.Exp, scale=0.5)
                kern = spool.tile([16, 512], f32)
                nc.scalar.activation(kern[:], d5[:], AF.Exp)
                ktp = pskt.tile([128, 64], f32)
                for c4 in range(4):
                    nc.tensor.transpose(
                        ktp[:, c4 * 16:(c4 + 1) * 16],
                        kern[:, c4 * 128:(c4 + 1) * 128],
                        idm[:],
                    )
                kt = ktpool.tile([128, 64], f32)
                nc.scalar.copy(kt[:], ktp[:])
                pf = pspfx.tile([128, 64], f32)
                nc.tensor.matmul(pf[:], ltr[:], kt[:],
                                 start=True, stop=True)
                sl = out_sb[:, p * 64:(p + 1) * 64]
                nc.vector.tensor_copy(sl, pf[:])
                nc.sync.dma_start(
                    pfx_ap[:, p * 64:(p + 1) * 64], sl)

            # 1-pair software stagger: pair p's post-dist work (kern chain,
            # transpose, prefix) is emitted behind pair p+1's load+dist
            # phase, so the in-order PE stream has ~17us of dist matmuls
            # between a pair's last accumulation and its transposes -- the
            # serial ACT kern chain never stalls PE.
            prev = None
            for p in range(NCHUNK // 2):
                dist = emit_dist_pair(p)
                if prev is not None:
                    emit_post(prev[0], prev[1])
                prev = (p, dist)
            emit_post(prev[0], prev[1])

    nc.compile()
    return nc


def _prep_inputs(h_t, cache_h, word_ids):
    h_t = np.asarray(h_t, dtype=np.float32)
    cache_h = np.asarray(cache_h, dtype=np.float32)
    word_ids = np.asarray(word_ids)

    order = np.argsort(word_ids, kind="stable")
    ws = np.asarray(word_ids[order], dtype=np.int64)

    # tile-major [core, 32*128, CHUNK]: tile t = (ch, c) holds d-slice
    # c*128..c*128+127 (partitions) x chunk rows (free), so each device
    # DMA is one contiguous 1 MiB read
    xt8 = np.ascontiguousarray(
        cache_h[order].reshape(NCORES, NCHUNK, CHUNK, 4, 128)
        .transpose(0, 1, 3, 4, 2)
        .reshape(NCORES, 32 * 128, CHUNK)
    ).astype(np.float16)

    hq = h_t.reshape(4, 128).T                      # [128, 4] quarters
    nh = np.ascontiguousarray(-hq).astype(np.float32)

    # oh column block u (u = sub*8 + g within a chunk pair) is a [128, 16]
    # lhsT routing group g of chunk 2p+sub to dist partition u
    oh = np.zeros((128, 256), np.float16)
    for u in range(16):
        oh[:, u * 16 + u] = 1.0
    idm = np.eye(16, dtype=np.float32)
    ltr = np.triu(np.ones((128, 128), np.float32))  # ltr[p, m] = (p <= m)

    in_maps = []
    for k in range(NCORES):
        in_maps.append({
            "xt": xt8[k], "nh": nh, "oh": oh, "idm": idm, "ltr": ltr,
        })
    return in_maps, ws


def _postprocess(pfx8, ws):
    """pfx8: [8, 128, BPC] within-(128)batch inclusive prefix sums, col
    order (pair, c4, sub, g); ws: sorted word_ids. Returns [1, V]."""
    i = np.arange(N_CACHE)
    k = i >> 15
    r = i & 32767
    ch = r >> 12
    rr = r & 4095
    g = rr >> 9
    rrr = rr & 511
    c4 = rrr >> 7
    p = rrr & 127
    col = (ch >> 1) * 64 + c4 * 16 + (ch & 1) * 8 + g

    P_wb = pfx8[k, p, col].astype(np.float64)
    # batch totals in global element order -> exclusive batch offsets
    T = pfx8[k[::128], 127, col[::128]].astype(np.float64)
    off = np.concatenate(([0.0], np.cumsum(T[:-1])))
    G = off[i >> 7] + P_wb          # global inclusive prefix at element i

    counts = np.bincount(ws, minlength=VOCAB)
    ends = np.cumsum(counts) - 1          # inclusive end index per vocab
    starts = ends - counts                # start-1 index per vocab
    Ge = G[np.maximum(ends, 0)]
    Gs = np.where(starts >= 0, G[np.maximum(starts, 0)], 0.0)
    cache_p = np.where(counts > 0, Ge - Gs, 0.0)

    m = cache_p.max()
    lse = m + np.log(np.exp(cache_p - m).sum())
    return (cache_p - lse).astype(np.float32)[None, :]


def kernel(h_t, cache_h, word_ids):
    from concourse.bass_utils import run_bass_kernel_spmd

    if "nc" not in _CACHE:
        _CACHE["nc"] = _build_program()
    nc = _CACHE["nc"]

    in_maps, ws = _prep_inputs(h_t, cache_h, word_ids)
    res = run_bass_kernel_spmd(nc, in_maps, list(range(NCORES)))

    pfx8 = np.stack([res.results[k]["pfx"] for k in range(NCORES)])
    return _postprocess(pfx8, ws)


# revision 37
# speedup vs baseline: 1.1260x; 1.1260x over previous
"""Trainium2 Bass kernel: cache-distance -> exp kernel -> vocab histogram -> log_softmax.

Math (per cache row i): kern_i = exp(||cache_h[i] - h_t|| / 0.2)
                        cache_p[v] = sum_{i: word_ids[i]==v} kern_i
                        out = log_softmax(cache_p)[None, :]

Device strategy (8 cores, cache sharded along N, DMA-roofline driven):
  - host sorts cache rows by word_id, so the vocab histogram becomes a
    segment-sum over consecutive elements; uploads the cache slice
    pre-transposed [D=512, 32768] in f16 (halves HBM traffic; validated
    numerically: f16-input rel err 2.8e-4 vs the 2e-2 gate)
  - squared distance via elementwise squares, split between two engines
    to stay under the DMA shadow (ACT 1x and DVE 4x/2x two-pass rates are
    ~balanced at a 13/19 quarter split):
      ACT quarters: (x + (-h))^2 via Square activation w/ per-partition bias
      DVE quarters: y = x + (-h) (tensor_scalar, 4x mode) then z = y*y
        (tensor_tensor, 2x mode) -- scalar_tensor_tensor would be one pass
        but only has a 1x uop (4480ns measured vs 3321ns for the pair)
  - PE reduces over D with one-hot-column [128,16] lhsT matmuls into a
    [16, 512] PSUM dist tile per CHUNK PAIR (2 x 4096 rows; pairing halves
    the ACT kern-chain instruction count and gives the software pipeline a
    full pair of PE work between cross-engine dependency edges)
  - ACT: kern = exp(exp(0.5*ln(25*d2)))  (Ln+Exp+Square share one table
    set; avoids Sqrt)
  - PE transposes kern into [128, 64] batch-major layout, then ONE
    lower-triangular [128,128] matmul per pair produces within-batch
    (128-element) inclusive prefix sums -- the entire scatter reduced to
    4 tiny matmuls per core
Host: combine per-core prefix tiles (f64 batch-offset cumsum), segment
  diffs at sorted-vocab boundaries, log_softmax (tiny, O(V)).
"""

import os
import sys

for _p in ("/root/.axon_site", "/root/.axon_site/_ro/trn_rl_repo",
           "/root/.axon_site/_ro/pypackages"):
    if os.path.isdir(_p) and _p not in sys.path:
        sys.path.append(_p)

import numpy as np

VOCAB = 50257
N_CACHE = 262144
D = 512
SMOOTH = 0.2
NCORES = 8
RPC = N_CACHE // NCORES        # 32768 rows per core
NCHUNK = 8
CHUNK = RPC // NCHUNK          # 4096 rows per chunk
BPC = RPC // 128               # 256 batches of 128 elements per core

_CACHE = {}


def _act_owns(ch, c):
    """13 of 32 (ch, c) quarter-tiles squared on ACT, the rest on DVE --
    balances ACT (1x + kern chain) against DVE (4x+2x two-pass). The last
    chunk gives ACT c=2 so the final DVE tile (c=3) squares in parallel
    with ACT instead of queueing behind a serial DVE backlog."""
    return c == 0 or (c == 2 and ch % 2 == 0) or (ch == 7 and c == 2)


def _patch_act_tables():
    """Restrict the activation table-set chooser to
    natural_log_exp_and_others (covers square/ln/exp/copy) so the whole
    kernel needs exactly one ACT_TABLE_LOAD instead of alternating between
    sets (~2.7us per reload). Set names/order are preserved so
    act_func_set_id indices stay valid."""
    import concourse.hw_specs as hw_specs
    import concourse.bacc as bacc

    if getattr(hw_specs.get_activation_tables, "_histkernel_patched", False):
        return
    orig = hw_specs.get_activation_tables

    def patched(module_arch):
        tabs = orig(module_arch)
        return {
            name: (fns if name == "natural_log_exp_and_others" else set())
            for name, fns in tabs.items()
        }

    patched._histkernel_patched = True
    hw_specs.get_activation_tables = patched
    bacc.get_activation_tables = patched


def _build_program():
    import concourse.bacc as bacc
    import concourse.tile as tile
    import concourse.mybir as mybir

    _patch_act_tables()

    f32, f16 = mybir.dt.float32, mybir.dt.float16
    AF = mybir.ActivationFunctionType
    ALU = mybir.AluOpType

    nc = bacc.Bacc("TRN2", target_bir_lowering=False, debug=False,
                   num_devices=NCORES)

    # tile-major layout: rows [t*128, (t+1)*128) hold quarter-tile
    # t = (ch, c)'s [128, 4096] block, so every DMA reads 1 MiB of
    # fully CONTIGUOUS DRAM (128 lines x 8KB, line stride == line size)
    xt_d = nc.dram_tensor("xt", [32 * 128, CHUNK], f16,
                          kind="ExternalInput")
    nh_d = nc.dram_tensor("nh", [128, 4], f32, kind="ExternalInput")
    oh_d = nc.dram_tensor("oh", [128, 256], f16, kind="ExternalInput")
    id_d = nc.dram_tensor("idm", [16, 16], f32, kind="ExternalInput")
    ltr_d = nc.dram_tensor("ltr", [128, 128], f32, kind="ExternalInput")
    pfx_d = nc.dram_tensor("pfx", [128, BPC], f32, kind="ExternalOutput")

    with tile.TileContext(nc) as tc:
        with (
            tc.tile_pool(name="const", bufs=1) as cpool,
            tc.tile_pool(name="x", bufs=14) as xpool,
            tc.tile_pool(name="sq", bufs=5) as sqpool,
            tc.tile_pool(name="y", bufs=3) as ypool,
            tc.tile_pool(name="s", bufs=4) as spool,
            tc.tile_pool(name="kt", bufs=2) as ktpool,
            tc.tile_pool(name="out", bufs=1) as opool,
            tc.tile_pool(name="psdist", bufs=4, space="PSUM") as psdist,
            tc.tile_pool(name="pskt", bufs=2, space="PSUM") as pskt,
            tc.tile_pool(name="pspfx", bufs=2, space="PSUM") as pspfx,
        ):
            # constants ride the scalar-engine HWDGE queue so the x-tile
            # stream on the sync queue starts issuing at t=0; nh goes
            # first on the sync queue as a tiny DGE-priming transfer
            nh = cpool.tile([128, 4], f32)
            nc.sync.dma_start(nh[:], nh_d.ap())
            oh = cpool.tile([128, 256], f16)
            nc.scalar.dma_start(oh[:], oh_d.ap())
            idm = cpool.tile([16, 16], f32)
            nc.scalar.dma_start(idm[:], id_d.ap())
            ltr = cpool.tile([128, 128], f32)
            nc.scalar.dma_start(ltr[:], ltr_d.ap())

            out_sb = opool.tile([128, BPC], f32)
            xt_ap = xt_d.ap()
            pfx_ap = pfx_d.ap()

            def emit_dist_pair(p):
                dist = psdist.tile([16, 512], f32)
                mm = 0
                for sub in range(2):
                    ch = 2 * p + sub
                    for c in range(4):
                        t = ch * 4 + c
                        x = xpool.tile([128, CHUNK], f16)
                        z = sqpool.tile([128, CHUNK], f16)
                        # the very last tile gates the kernel tail: split
                        # it in half so squaring overlaps its own DMA
                        H = CHUNK // 2
                        halves = ((0, H), (H, CHUNK)) if t == 31 \
                            else ((0, CHUNK),)
                        for lo, hi in halves:
                            nc.sync.dma_start(
                                x[:, lo:hi],
                                xt_ap[t * 128:(t + 1) * 128, lo:hi])
                        for hidx, (lo, hi) in enumerate(halves):
                            # split tile: ACT takes half 0, DVE half 1,
                            # so both halves square concurrently
                            on_act = (_act_owns(ch, c) if len(halves) == 1
                                      else hidx == 0)
                            if on_act:
                                nc.scalar.activation(
                                    z[:, lo:hi], x[:, lo:hi], AF.Square,
                                    bias=nh[:, c:c + 1])
                            else:
                                y = ypool.tile([128, CHUNK], f16)
                                nc.vector.tensor_scalar(
                                    y[:, lo:hi], x[:, lo:hi],
                                    nh[:, c:c + 1], None, ALU.add)
                                nc.vector.tensor_tensor(
                                    z[:, lo:hi], y[:, lo:hi], y[:, lo:hi],
                                    ALU.mult)
                        for g in range(8):
                            u = sub * 8 + g
                            nc.tensor.matmul(
                                dist[:],
                                oh[:, u * 16:(u + 1) * 16],
                                z[:, g * 512:(g + 1) * 512],
                                start=(mm == 0),
                                stop=(mm == 63),
                            )
                            mm += 1
                return dist

            def emit_post(p, dist):
                lg = spool.tile([16, 512], f32)
                nc.scalar.activation(lg[:], dist[:], AF.Ln, scale=25.0)
                d5 = spool.tile([16, 512], f32)
                nc.scalar.activation(d5[:], lg[:], AF.Exp, scale=0.5)
                kern = spool.tile([16, 512], f32)
                nc.scalar.activation(kern[:], d5[:], AF.Exp)
                ktp = pskt.tile([128, 64], f32)
                for c4 in range(4):
                    nc.tensor.transpose(
                        ktp[:, c4 * 16:(c4 + 1) * 16],
                        kern[:, c4 * 128:(c4 + 1) * 128],
                        idm[:],
                    )
                kt = ktpool.tile([128, 64], f32)
                nc.scalar.copy(kt[:], ktp[:])
                pf = pspfx.tile([128, 64], f32)
                nc.tensor.matmul(pf[:], ltr[:], kt[:],
                                 start=True, stop=True)
                sl = out_sb[:, p * 64:(p + 1) * 64]
                nc.vector.tensor_copy(sl, pf[:])
                nc.sync.dma_start(
                    pfx_ap[:, p * 64:(p + 1) * 64], sl)

            # 1-pair software stagger: pair p's post-dist work (kern chain,
            # transpose, prefix) is emitted behind pair p+1's load+dist
            # phase, so the in-order PE stream has ~17us of dist matmuls
            # between a pair's last accumulation and its transposes -- the
            # serial ACT kern chain never stalls PE.
            prev = None
            for p in range(NCHUNK // 2):
                dist = emit_dist_pair(p)
                if prev is not None:
                    emit_post(prev[0], prev[1])
                prev = (p, dist)
            emit_post(prev[0], prev[1])

    nc.compile()
    return nc


def _prep_inputs(h_t, cache_h, word_ids):
    h_t = np.asarray(h_t, dtype=np.float32)
    cache_h = np.asarray(cache_h, dtype=np.float32)
    word_ids = np.asarray(word_ids)

    order = np.argsort(word_ids, kind="stable")
    ws = np.asarray(word_ids[order], dtype=np.int64)

    # tile-major [core, 32*128, CHUNK]: tile t = (ch, c) holds d-slice
    # c*128..c*128+127 (partitions) x chunk rows (free), so each device
    # DMA is one contiguous 1 MiB read
    xt8 = np.ascontiguousarray(
        cache_h[order].reshape(NCORES, NCHUNK, CHUNK, 4, 128)
        .transpose(0, 1, 3, 4, 2)
        .reshape(NCORES, 32 * 128, CHUNK)
    ).astype(np.float16)

    hq = h_t.reshape(4, 128).T                      # [128, 4] quarters
    nh = np.ascontiguousarray(-hq).astype(np.float32)

    # oh column block u (u = sub*8 + g within a chunk pair) is a [128, 16]
    # lhsT routing group g of chunk 2p+sub to dist partition u
    oh = np.zeros((128, 256), np.float16)
    for u in range(16):
        oh[:, u * 16 + u] = 1.0
    idm = np.eye(16, dtype=np.float32)
    ltr = np.triu(np.ones((128, 128), np.float32))  # ltr[p, m] = (p <= m)

    in_maps = []
    for k in range(NCORES):
        in_maps.append({
            "xt": xt8[k], "nh": nh, "oh": oh, "idm": idm, "ltr": ltr,
        })
    return in_maps, ws


def _postprocess(pfx8, ws):
    """pfx8: [8, 128, BPC] within-(128)batch inclusive prefix sums, col
    order (pair, c4, sub, g); ws: sorted word_ids. Returns [1, V]."""
    i = np.arange(N_CACHE)
    k = i >> 15
    r = i & 32767
    ch = r >> 12
    rr = r & 4095
    g = rr >> 9
    rrr = rr & 511
    c4 = rrr >> 7
    p = rrr & 127
    col = (ch >> 1) * 64 + c4 * 16 + (ch & 1) * 8 + g

    P_wb = pfx8[k, p, col].astype(np.float64)
    # batch totals in global element order -> exclusive batch offsets
    T = pfx8[k[::128], 127, col[::128]].astype(np.float64)
    off = np.concatenate(([0.0], np.cumsum(T[:-1])))
    G = off[i >> 7] + P_wb          # global inclusive prefix at element i

    counts = np.bincount(ws, minlength=VOCAB)
    ends = np.cumsum(counts) - 1          # inclusive end index per vocab
    starts = ends - counts                # start-1 index per vocab
    Ge = G[np.maximum(ends, 0)]
    Gs = np.where(starts >= 0, G[np.maximum(starts, 0)], 0.0)
    cache_p = np.where(counts > 0, Ge - Gs, 0.0)

    m = cache_p.max()
    lse = m + np.log(np.exp(cache_p - m).sum())
    return (cache_p - lse).astype(np.float32)[None, :]


def kernel(h_t, cache_h, word_ids):
    from concourse.bass_utils import run_bass_kernel_spmd

    if "nc" not in _CACHE:
        _CACHE["nc"] = _build_program()
    nc = _CACHE["nc"]

    in_maps, ws = _prep_inputs(h_t, cache_h, word_ids)
    res = run_bass_kernel_spmd(nc, in_maps, list(range(NCORES)))

    pfx8 = np.stack([res.results[k]["pfx"] for k in range(NCORES)])
    return _postprocess(pfx8, ws)
